# revision 19
# baseline (speedup 1.0000x reference)
"""Fused multi-head-free attention + output projection for trn2, 8-core data parallel.

Per core (one batch element):
    scores = Q @ K^T / 32            [2048, 2048]
    E      = exp(scores)             (softmax max-subtraction skipped: scores ~ N(0,1))
    rowsum = sum_k E                 (via activation accum_out, free)
    P      = E * dropout_mask
    attn_r = P @ V                   (unnormalized)
    out    = (attn_r @ Wout^T) * (1/rowsum) + bout

All matmuls in bf16 (same PE rate as fp32r at N=512, halves SBUF, enables
DMA-xbar transposes which are 2-byte only). fp32 accumulation in PSUM.
Layouts: QT/KT = [d, s] via xbar transpose; V native [k, d]; P transposed
to [k, q] via xbar; PV produces attn^T [d, q] which feeds fc_out as lhsT.
"""

import math
import numpy as np
from contextlib import ExitStack

import concourse.bass as bass
import concourse.tile as tile
from concourse import mybir
from concourse import bass_utils

FP32 = mybir.dt.float32
BF16 = mybir.dt.bfloat16
AF = mybir.ActivationFunctionType

B, S, E = 8, 2048, 1024
N_CORES = 8
P = 128


def emit(ctx, tc, q, k, v, mask, wout, bout, out, inv_scale, s=S, e=E):
    nc = tc.nc
    NQ = s // P           # q tiles
    NK = s // P           # k chunks
    ND = e // P           # d chunks
    QB = s // 512         # q blocks (4 q-tiles each)
    KB = s // 512         # k blocks (512 wide)
    EB = e // 512         # e blocks

    const = ctx.enter_context(tc.tile_pool(name="const", bufs=1))
    persist = ctx.enter_context(tc.tile_pool(name="persist", bufs=1))
    stgf = ctx.enter_context(tc.tile_pool(name="stgf", bufs=2))
    stgb = ctx.enter_context(tc.tile_pool(name="stgb", bufs=2))
    qtw_pool = ctx.enter_context(tc.tile_pool(name="qtw", bufs=2))
    epool = ctx.enter_context(tc.tile_pool(name="epool", bufs=2))
    ppool = ctx.enter_context(tc.tile_pool(name="ppool", bufs=2))
    mpool = ctx.enter_context(tc.tile_pool(name="mpool", bufs=2))
    ptpool = ctx.enter_context(tc.tile_pool(name="ptpool", bufs=2))
    atpool = ctx.enter_context(tc.tile_pool(name="atpool", bufs=2))
    opool = ctx.enter_context(tc.tile_pool(name="opool", bufs=2))
    small = ctx.enter_context(tc.tile_pool(name="small", bufs=2))
    ps_s = ctx.enter_context(tc.tile_pool(name="ps_s", bufs=2, space="PSUM"))
    ps_a = ctx.enter_context(tc.tile_pool(name="ps_a", bufs=2, space="PSUM"))
    ps_o = ctx.enter_context(tc.tile_pool(name="ps_o", bufs=2, space="PSUM"))

    # single big tensors: one xbar transpose writes a whole [P, ND, 128]
    # 3D slice, so each consumer tile has few writers and each transpose has
    # exactly one (compute-engine) producer dep -> fits the HWDGE 1-wait limit
    KTa = persist.tile([P, ND * s], BF16, tag="kta", name="kta")   # [d, k] blocks
    WTa = persist.tile([P, ND * e], BF16, tag="wta", name="wta")   # [d, e] blocks
    VN = [persist.tile([P, e], BF16, tag=f"v{c}", name=f"v{c}") for c in range(NK)]
    bb = const.tile([P, e], BF16, name="bb")

    def kt_out(c):   # KTa columns d*s + c*128 .. (3D: [P, d, 128])
        return KTa[:].rearrange("p (d i) -> p d i", i=s)[:, :, c * P:(c + 1) * P]

    def wt_out(c):
        return WTa[:].rearrange("p (d i) -> p d i", i=e)[:, :, c * P:(c + 1) * P]

    def load_cast(dram, c, tag):
        # SWDGE load (multi-wait capable) + DVE cast: every xbar transpose
        # then has a single DVE producer dep (merges with memset WAW waits)
        sf = stgf.tile([P, e], FP32, tag="sf", name=f"sf_{tag}{c}")
        nc.gpsimd.dma_start(out=sf[:], in_=dram[c * P:(c + 1) * P, :])
        sb = stgb.tile([P, e], BF16, tag="sb", name=f"sb_{tag}{c}")
        nc.vector.tensor_copy(sb[:], sf[:])
        return sb

    # Dummy transpose with zero data deps (DRAM source): absorbs the one-time
    # copy->transpose xbar-mode serialization wait so every later transpose
    # on the (transpose-only) SP ring carries exactly one sync wait.
    junk = const.tile([P, P], mybir.dt.uint16, name="junk")
    nc.sync.dma_start(out=junk[:], in_=q[0:P, 0:64].bitcast(mybir.dt.uint16),
                      transpose=True)

    masks = {}

    def load_mask(qtg):
        mt = mpool.tile([P, s], BF16, tag="m", name=f"m{qtg}")
        nc.gpsimd.dma_start(out=mt[:], in_=mask[qtg * P:(qtg + 1) * P, :])
        masks[qtg] = mt

    qtws = {}

    def prepare_qtw(qb):
        # Q^T window for one q-block: [P, d, 512] built by 4 transposes.
        # The memset is the generation's first writer: it absorbs the WAR
        # waits vs last generation's PE readers on a multi-wait-capable
        # engine, keeping the (1-wait-limited) xbar transposes to one dep.
        qtwt = qtw_pool.tile([P, ND * 512], BF16, tag="qtw", name=f"qtw{qb}")
        nc.vector.memset(qtwt[:], 0.0)
        qtw3 = qtwt[:].rearrange("p (d i) -> p d i", i=512)
        for cq in range(4):
            sb = load_cast(q, qb * 4 + cq, "q")
            nc.sync.dma_start(out=qtw3[:, :, cq * P:(cq + 1) * P],
                              in_=sb[:], transpose=True)
        qtws[qb] = qtwt

    # SWDGE FIFO order is execution order for loads: K first (gates all QK),
    # first masks interleaved mid-K, first two Q windows next, then V (needed
    # at first PV, ~35us in), then W (first FC, ~50us in).
    for c in range(NK):
        sb = load_cast(k, c, "k")
        nc.sync.dma_start(out=kt_out(c), in_=sb[:], transpose=True)
        if c == 7:
            load_mask(0)
    prepare_qtw(0)
    load_mask(1)
    if QB > 1:
        prepare_qtw(1)
    for c in range(NK):
        nc.gpsimd.dma_start(out=VN[c][:], in_=v[c * P:(c + 1) * P, :])
    for c in range(ND):
        sb = load_cast(wout, c, "w")
        nc.sync.dma_start(out=wt_out(c), in_=sb[:], transpose=True)
    bout_bcast = bass.AP(tensor=bout.tensor, offset=bout.offset,
                         ap=[[0, P]] + list(bout.ap))
    nc.gpsimd.dma_start(out=bb[:], in_=bout_bcast)

    for qb in range(QB):
        if qb not in qtws:
            prepare_qtw(qb)
        qtwt = qtws[qb]
        pta = ptpool.tile([P, NK * 512], BF16, tag="pta", name=f"pta{qb}")
        nc.vector.memset(pta[:], 0.0)
        pta3 = pta[:].rearrange("p (c i) -> p c i", i=512)
        recips = []
        for qt in range(4):
            qtg = qb * 4 + qt
            et = epool.tile([P, s], BF16, tag="e", name=f"e{qtg}")
            rs4 = small.tile([P, KB], FP32, tag=f"rs{qt}", name=f"rs{qtg}")
            for kb2 in range(KB // 2):
                pss = ps_s.tile([P, 1024], FP32, tag="ps_s", name=f"pss{qtg}_{kb2}")
                for h in range(2):
                    kb = kb2 * 2 + h
                    for d in range(ND):
                        nc.tensor.matmul(
                            pss[:, h * 512:(h + 1) * 512],
                            qtwt[:, d * 512 + qt * P: d * 512 + (qt + 1) * P],
                            KTa[:, d * s + kb * 512: d * s + (kb + 1) * 512],
                            start=(d == 0), stop=(d == ND - 1))
                nc.scalar.activation(et[:, kb2 * 1024:(kb2 + 1) * 1024], pss[:],
                                     AF.Exp, bias=0.0, scale=inv_scale,
                                     accum_out=rs4[:, kb2:kb2 + 1])
            rs1 = small.tile([P, 1], FP32, tag=f"rs1_{qt}", name=f"rs1_{qtg}")
            nc.vector.reduce_sum(rs1[:], rs4[:, 0:KB // 2], axis=mybir.AxisListType.X)
            rec = small.tile([P, 1], FP32, tag=f"rec{qt}", name=f"rec{qtg}")
            nc.vector.reciprocal(rec[:], rs1[:])
            recips.append(rec)
            # mask streams on SWDGE (cast to bf16); P = E*mask into a fresh
            # tile so the P'T transpose has a single DVE producer dep
            mt = mpool.tile([P, s], BF16, tag="m", name=f"m{qtg}")
            nc.gpsimd.dma_start(out=mt[:], in_=mask[qtg * P:(qtg + 1) * P, :])
            pt2 = ppool.tile([P, s], BF16, tag="p2", name=f"p2_{qtg}")
            nc.vector.tensor_mul(pt2[:], et[:], mt[:])
            nc.sync.dma_start(out=pta3[:, :, qt * P:(qt + 1) * P],
                              in_=pt2[:], transpose=True)
        ats = [atpool.tile([P, 512], BF16, tag=f"at{d}", name=f"at_{qb}_{d}")
               for d in range(ND)]
        for d in range(ND):
            psa = ps_a.tile([P, 512], FP32, tag="ps_a", name=f"psa{qb}_{d}")
            for c in range(NK):
                nc.tensor.matmul(psa[:], VN[c][:, d * P:(d + 1) * P],
                                 pta[:, c * 512:(c + 1) * 512],
                                 start=(c == 0), stop=(c == NK - 1))
            nc.scalar.activation(ats[d][:], psa[:], AF.Copy, bias=0.0, scale=1.0)
        for qt in range(4):
            qtg = qb * 4 + qt
            osb = opool.tile([P, e], FP32, tag="osb", name=f"osb{qtg}")
            for eb in range(EB):
                pso = ps_o.tile([P, 512], FP32, tag="ps_o", name=f"pso{qtg}_{eb}")
                for d in range(ND):
                    nc.tensor.matmul(pso[:], ats[d][:, qt * P:(qt + 1) * P],
                                     WTa[:, d * e + eb * 512: d * e + (eb + 1) * 512],
                                     start=(d == 0), stop=(d == ND - 1))
                nc.scalar.activation(osb[:, eb * 512:(eb + 1) * 512], pso[:],
                                     AF.Copy, bias=0.0, scale=recips[qt][:, 0:1])
            nc.vector.tensor_add(osb[:], osb[:], bb[:])
            nc.gpsimd.dma_start(out=out[qtg * P:(qtg + 1) * P, :], in_=osb[:])


_DMA_TYPES = ("InstDmaTransposeAnt", "InstDMACopy")


def _offload_hwdge_waits(nc):
    """walrus's per-instruction sync-wait slots are tiny (1 for DMA structs,
    ~2 for compute structs). Move excess waits onto ENGINE_NOPs spliced just
    before the instruction on the same engine stream — the sequencer blocks
    on the nops' waits in order, then issues the instruction; semantics
    unchanged."""
    eng_map = {"EngineType.SP": nc.sync, "EngineType.Activation": nc.scalar,
               "EngineType.Pool": nc.gpsimd, "EngineType.PE": nc.tensor,
               "EngineType.DVE": nc.vector}
    for bb in nc.main_func.blocks:
        insts = list(bb.instructions)
        out = []
        for ins in insts:
            si = getattr(ins, "sync_info", None)
            eng = eng_map.get(str(getattr(ins, "engine", None)))
            if si is not None and eng is not None and si.on_wait:
                cap = 1
                if len(si.on_wait) > cap:
                    keep = si.on_wait[:cap] if cap > 0 else []
                    excess = si.on_wait[cap:]
                    opc = nc.isa.Opcode.NEURON_ISA_TPB_OPCODE_NOP
                    for w in excess:
                        nop = eng._isa(opc, {})
                        nop.engine = ins.engine
                        nop.sync_info = mybir.SyncInfo(on_wait=[w], on_update=[])
                        nc.inst_map[nop.name] = nop
                        out.append(nop)
                    ins.sync_info.on_wait = list(keep)
            out.append(ins)
        bb.instructions[:] = out


def build(inv_scale_factor=32.0, s=S, e=E, repeat=1):
    nc = bass.Bass("TRN2", target_bir_lowering=False, debug=False,
                   num_devices=N_CORES)
    q = nc.dram_tensor("q", [s, e], FP32, kind="ExternalInput").ap()
    k = nc.dram_tensor("k", [s, e], FP32, kind="ExternalInput").ap()
    v = nc.dram_tensor("v", [s, e], FP32, kind="ExternalInput").ap()
    mask = nc.dram_tensor("mask", [s, s], FP32, kind="ExternalInput").ap()
    wout = nc.dram_tensor("wout", [e, e], FP32, kind="ExternalInput").ap()
    bout = nc.dram_tensor("bout", [e], FP32, kind="ExternalInput").ap()
    out = nc.dram_tensor("out", [s, e], FP32, kind="ExternalOutput").ap()
    with tile.TileContext(nc) as tc:
        for _ in range(repeat):
            with ExitStack() as ctx:
                emit(ctx, tc, q, k, v, mask, wout, bout, out,
                     1.0 / float(inv_scale_factor), s=s, e=e)
    _offload_hwdge_waits(nc)
    return nc


def make_in_maps(query, key, value, dropout_mask, Wout, bout):
    f32 = np.float32
    Wout = np.ascontiguousarray(Wout, dtype=f32)
    bvec = np.ascontiguousarray(bout, dtype=f32)
    return [{
        "q": np.ascontiguousarray(query[i], dtype=f32),
        "k": np.ascontiguousarray(key[i], dtype=f32),
        "v": np.ascontiguousarray(value[i], dtype=f32),
        "mask": np.ascontiguousarray(dropout_mask[i], dtype=f32),
        "wout": Wout,
        "bout": bvec,
    } for i in range(N_CORES)]


def run(inputs, trace=False, **trace_kwargs):
    nc = build(float(inputs.get("inv_scale_factor", 32)))
    in_maps = make_in_maps(inputs["query"], inputs["key"], inputs["value"],
                           inputs["dropout_mask"], inputs["Wout"], inputs["bout"])
    res = bass_utils.run_bass_kernel_spmd(
        nc, in_maps, core_ids=list(range(N_CORES)), trace=trace, **trace_kwargs)
    out = np.stack([np.asarray(res.results[i]["out"]) for i in range(N_CORES)])
    return out.astype(np.float32), res


def kernel(query, key, value, dropout_mask, Wout, bout, inv_scale_factor=32):
    out, _ = run(dict(query=query, key=key, value=value,
                      dropout_mask=dropout_mask, Wout=Wout, bout=bout,
                      inv_scale_factor=inv_scale_factor))
    return out


# revision 20
# speedup vs baseline: 2.7345x; 2.7345x over previous
"""Fused multi-head-free attention + output projection for trn2, 8-core data parallel.

Per core (one batch element):
    scores = Q @ K^T / 32            [2048, 2048]
    E      = exp(scores)             (softmax max-subtraction skipped: scores ~ N(0,1))
    rowsum = sum_k E                 (via activation accum_out, free)
    P      = E * dropout_mask
    attn_r = P @ V                   (unnormalized)
    out    = (attn_r @ Wout^T) * (1/rowsum) + bout

All matmuls in bf16 (same PE rate as fp32r at N=512, halves SBUF, enables
DMA-xbar transposes which are 2-byte only). fp32 accumulation in PSUM.
Layouts: QT/KT = [d, s] via xbar transpose; V native [k, d]; P transposed
to [k, q] via xbar; PV produces attn^T [d, q] which feeds fc_out as lhsT.
"""

import math
import numpy as np
from contextlib import ExitStack

import concourse.bass as bass
import concourse.tile as tile
from concourse import mybir
from concourse import bass_utils

FP32 = mybir.dt.float32
BF16 = mybir.dt.bfloat16
AF = mybir.ActivationFunctionType

B, S, E = 8, 2048, 1024
N_CORES = 8
P = 128


def emit(ctx, tc, q, k, v, mask, wout, bout, out, inv_scale, s=S, e=E):
    nc = tc.nc
    NQ = s // P           # q tiles
    NK = s // P           # k chunks
    ND = e // P           # d chunks
    QB = s // 512         # q blocks (4 q-tiles each)
    KB = s // 512         # k blocks (512 wide)
    EB = e // 512         # e blocks

    const = ctx.enter_context(tc.tile_pool(name="const", bufs=1))
    persist = ctx.enter_context(tc.tile_pool(name="persist", bufs=1))
    stgf = ctx.enter_context(tc.tile_pool(name="stgf", bufs=2))
    stgb = ctx.enter_context(tc.tile_pool(name="stgb", bufs=2))
    qtw_pool = ctx.enter_context(tc.tile_pool(name="qtw", bufs=2))
    epool = ctx.enter_context(tc.tile_pool(name="epool", bufs=2))
    ppool = ctx.enter_context(tc.tile_pool(name="ppool", bufs=2))
    mpool = ctx.enter_context(tc.tile_pool(name="mpool", bufs=2))
    ptpool = ctx.enter_context(tc.tile_pool(name="ptpool", bufs=2))
    atpool = ctx.enter_context(tc.tile_pool(name="atpool", bufs=2))
    opool = ctx.enter_context(tc.tile_pool(name="opool", bufs=2))
    small = ctx.enter_context(tc.tile_pool(name="small", bufs=2))
    ps_s = ctx.enter_context(tc.tile_pool(name="ps_s", bufs=2, space="PSUM"))
    ps_a = ctx.enter_context(tc.tile_pool(name="ps_a", bufs=2, space="PSUM"))
    ps_o = ctx.enter_context(tc.tile_pool(name="ps_o", bufs=2, space="PSUM"))

    # single big tensors: one xbar transpose writes a whole [P, ND, 128]
    # 3D slice, so each consumer tile has few writers and each transpose has
    # exactly one (compute-engine) producer dep -> fits the HWDGE 1-wait limit
    KTa = persist.tile([P, ND * s], BF16, tag="kta", name="kta")   # [d, k] blocks
    WTa = persist.tile([P, ND * e], BF16, tag="wta", name="wta")   # [d, e] blocks
    VN = [persist.tile([P, e], BF16, tag=f"v{c}", name=f"v{c}") for c in range(NK)]
    bb = const.tile([P, e], BF16, name="bb")

    def kt_out(c, dlo, dhi):   # KTa columns d*s + c*128 .. (3D: [P, d, 128])
        return KTa[:].rearrange("p (d i) -> p d i", i=s)[:, dlo:dhi, c * P:(c + 1) * P]

    def wt_out(c, dlo, dhi):
        return WTa[:].rearrange("p (d i) -> p d i", i=e)[:, dlo:dhi, c * P:(c + 1) * P]

    def load_cast(dram, c, tag):
        # SWDGE load (multi-wait capable) + DVE cast: every xbar transpose
        # then has a single DVE producer dep (merges with memset WAW waits)
        sf = stgf.tile([P, e], FP32, tag="sf", name=f"sf_{tag}{c}")
        nc.gpsimd.dma_start(out=sf[:], in_=dram[c * P:(c + 1) * P, :])
        sb = stgb.tile([P, e], BF16, tag="sb", name=f"sb_{tag}{c}")
        nc.vector.tensor_copy(sb[:], sf[:])
        return sb

    # Dummy transpose with zero data deps (DRAM source): absorbs the one-time
    # copy->transpose xbar-mode serialization wait so every later transpose
    # on the (transpose-only) SP ring carries exactly one sync wait.
    junk = const.tile([P, P], mybir.dt.uint16, name="junk")
    nc.sync.dma_start(out=junk[:], in_=q[0:P, 0:64].bitcast(mybir.dt.uint16),
                      transpose=True)

    masks = {}

    def load_mask(qtg):
        mt = mpool.tile([P, s], BF16, tag="m", name=f"m{qtg}")
        nc.gpsimd.dma_start(out=mt[:], in_=mask[qtg * P:(qtg + 1) * P, :])
        masks[qtg] = mt

    qtws = {}

    def prepare_qtw(qb):
        # Q^T window for one q-block: [P, d, 512] built by 4 transposes.
        # The memset is the generation's first writer: it absorbs the WAR
        # waits vs last generation's PE readers on a multi-wait-capable
        # engine, keeping the (1-wait-limited) xbar transposes to one dep.
        qtwt = qtw_pool.tile([P, ND * 512], BF16, tag="qtw", name=f"qtw{qb}")
        nc.vector.memset(qtwt[:], 0.0)
        qtw3 = qtwt[:].rearrange("p (d i) -> p d i", i=512)
        for cq in range(4):
            sb = load_cast(q, qb * 4 + cq, "q")
            for hh in range(2):
                nc.sync.dma_start(
                    out=qtw3[:, hh * ND // 2:(hh + 1) * ND // 2,
                             cq * P:(cq + 1) * P],
                    in_=sb[:, hh * e // 2:(hh + 1) * e // 2], transpose=True)
        qtws[qb] = qtwt

    # SWDGE FIFO order is execution order for loads: K first (gates all QK),
    # first masks interleaved mid-K, first two Q windows next, then V (needed
    # at first PV, ~35us in), then W (first FC, ~50us in).
    for c in range(NK):
        sb = load_cast(k, c, "k")
        for hh in range(2):
            nc.sync.dma_start(out=kt_out(c, hh * ND // 2, (hh + 1) * ND // 2),
                              in_=sb[:, hh * e // 2:(hh + 1) * e // 2],
                              transpose=True)
        if c == 7:
            load_mask(0)
    prepare_qtw(0)
    load_mask(1)
    if QB > 1:
        prepare_qtw(1)
    for c in range(NK):
        nc.gpsimd.dma_start(out=VN[c][:], in_=v[c * P:(c + 1) * P, :])
    for c in range(ND):
        sb = load_cast(wout, c, "w")
        for hh in range(2):
            nc.sync.dma_start(out=wt_out(c, hh * ND // 2, (hh + 1) * ND // 2),
                              in_=sb[:, hh * e // 2:(hh + 1) * e // 2],
                              transpose=True)
    bout_bcast = bass.AP(tensor=bout.tensor, offset=bout.offset,
                         ap=[[0, P]] + list(bout.ap))
    nc.gpsimd.dma_start(out=bb[:], in_=bout_bcast)

    for qb in range(QB):
        if qb not in qtws:
            prepare_qtw(qb)
        qtwt = qtws[qb]
        pta = ptpool.tile([P, NK * 512], BF16, tag="pta", name=f"pta{qb}")
        nc.vector.memset(pta[:], 0.0)
        pta3 = pta[:].rearrange("p (c i) -> p c i", i=512)
        recips = []
        for qt in range(4):
            qtg = qb * 4 + qt
            et = epool.tile([P, s], BF16, tag="e", name=f"e{qtg}")
            rs4 = small.tile([P, KB], FP32, tag=f"rs{qt}", name=f"rs{qtg}")
            for kb2 in range(KB // 2):
                pss = ps_s.tile([P, 1024], FP32, tag="ps_s", name=f"pss{qtg}_{kb2}")
                for h in range(2):
                    kb = kb2 * 2 + h
                    for d in range(ND):
                        nc.tensor.matmul(
                            pss[:, h * 512:(h + 1) * 512],
                            qtwt[:, d * 512 + qt * P: d * 512 + (qt + 1) * P],
                            KTa[:, d * s + kb * 512: d * s + (kb + 1) * 512],
                            start=(d == 0), stop=(d == ND - 1))
                nc.scalar.activation(et[:, kb2 * 1024:(kb2 + 1) * 1024], pss[:],
                                     AF.Exp, bias=0.0, scale=inv_scale,
                                     accum_out=rs4[:, kb2:kb2 + 1])
            rs1 = small.tile([P, 1], FP32, tag=f"rs1_{qt}", name=f"rs1_{qtg}")
            nc.vector.reduce_sum(rs1[:], rs4[:, 0:KB // 2], axis=mybir.AxisListType.X)
            rec = small.tile([P, 1], FP32, tag=f"rec{qt}", name=f"rec{qtg}")
            nc.vector.reciprocal(rec[:], rs1[:])
            recips.append(rec)
            # mask streams on SWDGE (cast to bf16); P = E*mask into a fresh
            # tile so the P'T transpose has a single DVE producer dep
            mt = mpool.tile([P, s], BF16, tag="m", name=f"m{qtg}")
            nc.gpsimd.dma_start(out=mt[:], in_=mask[qtg * P:(qtg + 1) * P, :])
            pt2 = ppool.tile([P, s], BF16, tag="p2", name=f"p2_{qtg}")
            nc.vector.tensor_mul(pt2[:], et[:], mt[:])
            for jj in range(4):
                nc.sync.dma_start(
                    out=pta3[:, jj * NK // 4:(jj + 1) * NK // 4,
                             qt * P:(qt + 1) * P],
                    in_=pt2[:, jj * s // 4:(jj + 1) * s // 4], transpose=True)
        ats = [atpool.tile([P, 512], BF16, tag=f"at{d}", name=f"at_{qb}_{d}")
               for d in range(ND)]
        for d in range(ND):
            psa = ps_a.tile([P, 512], FP32, tag="ps_a", name=f"psa{qb}_{d}")
            for c in range(NK):
                nc.tensor.matmul(psa[:], VN[c][:, d * P:(d + 1) * P],
                                 pta[:, c * 512:(c + 1) * 512],
                                 start=(c == 0), stop=(c == NK - 1))
            nc.scalar.activation(ats[d][:], psa[:], AF.Copy, bias=0.0, scale=1.0)
        for qt in range(4):
            qtg = qb * 4 + qt
            osb = opool.tile([P, e], FP32, tag="osb", name=f"osb{qtg}")
            for eb in range(EB):
                pso = ps_o.tile([P, 512], FP32, tag="ps_o", name=f"pso{qtg}_{eb}")
                for d in range(ND):
                    nc.tensor.matmul(pso[:], ats[d][:, qt * P:(qt + 1) * P],
                                     WTa[:, d * e + eb * 512: d * e + (eb + 1) * 512],
                                     start=(d == 0), stop=(d == ND - 1))
                nc.scalar.activation(osb[:, eb * 512:(eb + 1) * 512], pso[:],
                                     AF.Copy, bias=0.0, scale=recips[qt][:, 0:1])
            nc.vector.tensor_add(osb[:], osb[:], bb[:])
            nc.gpsimd.dma_start(out=out[qtg * P:(qtg + 1) * P, :], in_=osb[:])


_DMA_TYPES = ("InstDmaTransposeAnt", "InstDMACopy")


def _offload_hwdge_waits(nc):
    """walrus's per-instruction sync-wait slots are tiny (1 for DMA structs,
    ~2 for compute structs). Move excess waits onto ENGINE_NOPs spliced just
    before the instruction on the same engine stream — the sequencer blocks
    on the nops' waits in order, then issues the instruction; semantics
    unchanged."""
    eng_map = {"EngineType.SP": nc.sync, "EngineType.Activation": nc.scalar,
               "EngineType.Pool": nc.gpsimd, "EngineType.PE": nc.tensor,
               "EngineType.DVE": nc.vector}
    for bb in nc.main_func.blocks:
        insts = list(bb.instructions)
        out = []
        for ins in insts:
            si = getattr(ins, "sync_info", None)
            eng = eng_map.get(str(getattr(ins, "engine", None)))
            if si is not None and eng is not None and si.on_wait:
                cap = 1
                if len(si.on_wait) > cap:
                    keep = si.on_wait[:cap] if cap > 0 else []
                    excess = si.on_wait[cap:]
                    opc = nc.isa.Opcode.NEURON_ISA_TPB_OPCODE_NOP
                    for w in excess:
                        nop = eng._isa(opc, {})
                        nop.engine = ins.engine
                        nop.sync_info = mybir.SyncInfo(on_wait=[w], on_update=[])
                        nc.inst_map[nop.name] = nop
                        out.append(nop)
                    ins.sync_info.on_wait = list(keep)
            out.append(ins)
        bb.instructions[:] = out


def build(inv_scale_factor=32.0, s=S, e=E, repeat=1):
    nc = bass.Bass("TRN2", target_bir_lowering=False, debug=False,
                   num_devices=N_CORES)
    q = nc.dram_tensor("q", [s, e], FP32, kind="ExternalInput").ap()
    k = nc.dram_tensor("k", [s, e], FP32, kind="ExternalInput").ap()
    v = nc.dram_tensor("v", [s, e], FP32, kind="ExternalInput").ap()
    mask = nc.dram_tensor("mask", [s, s], FP32, kind="ExternalInput").ap()
    wout = nc.dram_tensor("wout", [e, e], FP32, kind="ExternalInput").ap()
    bout = nc.dram_tensor("bout", [e], FP32, kind="ExternalInput").ap()
    out = nc.dram_tensor("out", [s, e], FP32, kind="ExternalOutput").ap()
    with tile.TileContext(nc) as tc:
        for _ in range(repeat):
            with ExitStack() as ctx:
                emit(ctx, tc, q, k, v, mask, wout, bout, out,
                     1.0 / float(inv_scale_factor), s=s, e=e)
    _offload_hwdge_waits(nc)
    return nc


def make_in_maps(query, key, value, dropout_mask, Wout, bout):
    f32 = np.float32
    Wout = np.ascontiguousarray(Wout, dtype=f32)
    bvec = np.ascontiguousarray(bout, dtype=f32)
    return [{
        "q": np.ascontiguousarray(query[i], dtype=f32),
        "k": np.ascontiguousarray(key[i], dtype=f32),
        "v": np.ascontiguousarray(value[i], dtype=f32),
        "mask": np.ascontiguousarray(dropout_mask[i], dtype=f32),
        "wout": Wout,
        "bout": bvec,
    } for i in range(N_CORES)]


def run(inputs, trace=False, **trace_kwargs):
    nc = build(float(inputs.get("inv_scale_factor", 32)))
    in_maps = make_in_maps(inputs["query"], inputs["key"], inputs["value"],
                           inputs["dropout_mask"], inputs["Wout"], inputs["bout"])
    res = bass_utils.run_bass_kernel_spmd(
        nc, in_maps, core_ids=list(range(N_CORES)), trace=trace, **trace_kwargs)
    out = np.stack([np.asarray(res.results[i]["out"]) for i in range(N_CORES)])
    return out.astype(np.float32), res


def kernel(query, key, value, dropout_mask, Wout, bout, inv_scale_factor=32):
    out, _ = run(dict(query=query, key=key, value=value,
                      dropout_mask=dropout_mask, Wout=Wout, bout=bout,
                      inv_scale_factor=inv_scale_factor))
    return out


# revision 21
# speedup vs baseline: 5.3256x; 1.9476x over previous
"""Fused multi-head-free attention + output projection for trn2, 8-core data parallel.

Per core (one batch element):
    scores = Q @ K^T / 32            [2048, 2048]
    E      = exp(scores)             (softmax max-subtraction skipped: scores ~ N(0,1))
    rowsum = sum_k E                 (via activation accum_out, free)
    P      = E * dropout_mask
    attn_r = P @ V                   (unnormalized)
    out    = (attn_r @ Wout^T) * (1/rowsum) + bout

All matmuls in bf16 (same PE rate as fp32r at N=512, halves SBUF, enables
DMA-xbar transposes which are 2-byte only). fp32 accumulation in PSUM.
Layouts: QT/KT = [d, s] via xbar transpose; V native [k, d]; P transposed
to [k, q] via xbar; PV produces attn^T [d, q] which feeds fc_out as lhsT.
"""

import math
import numpy as np
from contextlib import ExitStack

import concourse.bass as bass
import concourse.tile as tile
from concourse import mybir
from concourse import bass_utils

FP32 = mybir.dt.float32
BF16 = mybir.dt.bfloat16
AF = mybir.ActivationFunctionType

B, S, E = 8, 2048, 1024
N_CORES = 8
P = 128


def emit(ctx, tc, q, k, v, mask, wout, bout, out, inv_scale, s=S, e=E):
    nc = tc.nc
    NQ = s // P           # q tiles
    NK = s // P           # k chunks
    ND = e // P           # d chunks
    QB = s // 512         # q blocks (4 q-tiles each)
    KB = s // 512         # k blocks (512 wide)
    EB = e // 512         # e blocks

    const = ctx.enter_context(tc.tile_pool(name="const", bufs=1))
    persist = ctx.enter_context(tc.tile_pool(name="persist", bufs=1))
    stgf = ctx.enter_context(tc.tile_pool(name="stgf", bufs=2))
    stgb = ctx.enter_context(tc.tile_pool(name="stgb", bufs=2))
    qtw_pool = ctx.enter_context(tc.tile_pool(name="qtw", bufs=2))
    epool = ctx.enter_context(tc.tile_pool(name="epool", bufs=2))
    ppool = ctx.enter_context(tc.tile_pool(name="ppool", bufs=2))
    mpool = ctx.enter_context(tc.tile_pool(name="mpool", bufs=2))
    ptpool = ctx.enter_context(tc.tile_pool(name="ptpool", bufs=2))
    atpool = ctx.enter_context(tc.tile_pool(name="atpool", bufs=2))
    opool = ctx.enter_context(tc.tile_pool(name="opool", bufs=2))
    small = ctx.enter_context(tc.tile_pool(name="small", bufs=2))
    ps_s = ctx.enter_context(tc.tile_pool(name="ps_s", bufs=2, space="PSUM"))
    ps_a = ctx.enter_context(tc.tile_pool(name="ps_a", bufs=2, space="PSUM"))
    ps_o = ctx.enter_context(tc.tile_pool(name="ps_o", bufs=2, space="PSUM"))

    # single big tensors: one xbar transpose writes a whole [P, ND, 128]
    # 3D slice, so each consumer tile has few writers and each transpose has
    # exactly one (compute-engine) producer dep -> fits the HWDGE 1-wait limit
    KTa = persist.tile([P, ND * s], BF16, tag="kta", name="kta")   # [d, k] blocks
    WTa = persist.tile([P, ND * e], BF16, tag="wta", name="wta")   # [d, e] blocks
    VN = [persist.tile([P, e], BF16, tag=f"v{c}", name=f"v{c}") for c in range(NK)]
    bb = const.tile([P, e], BF16, name="bb")

    def kt_out(c, dlo, dhi):   # KTa columns d*s + c*128 .. (3D: [P, d, 128])
        return KTa[:].rearrange("p (d i) -> p d i", i=s)[:, dlo:dhi, c * P:(c + 1) * P]

    def wt_out(c, dlo, dhi):
        return WTa[:].rearrange("p (d i) -> p d i", i=e)[:, dlo:dhi, c * P:(c + 1) * P]

    def load_cast(dram, c, tag):
        # SWDGE load (multi-wait capable) + DVE cast: every xbar transpose
        # then has a single DVE producer dep (merges with memset WAW waits)
        sf = stgf.tile([P, e], FP32, tag="sf", name=f"sf_{tag}{c}")
        nc.gpsimd.dma_start(out=sf[:], in_=dram[c * P:(c + 1) * P, :])
        sb = stgb.tile([P, e], BF16, tag="sb", name=f"sb_{tag}{c}")
        nc.vector.tensor_copy(sb[:], sf[:])
        return sb

    # Dummy transpose with zero data deps (DRAM source): absorbs the one-time
    # copy->transpose xbar-mode serialization wait so every later transpose
    # on the (transpose-only) SP ring carries exactly one sync wait.
    junk = const.tile([P, P], mybir.dt.uint16, name="junk")
    nc.sync.dma_start(out=junk[:], in_=q[0:P, 0:64].bitcast(mybir.dt.uint16),
                      transpose=True)

    masks = {}

    def load_mask(qtg):
        mt = mpool.tile([P, s], BF16, tag="m", name=f"m{qtg}")
        nc.gpsimd.dma_start(out=mt[:], in_=mask[qtg * P:(qtg + 1) * P, :])
        masks[qtg] = mt

    qtws = {}

    def prepare_qtw(qb):
        # Q^T window for one q-block: [P, d, 512] built by 4 transposes.
        # The memset is the generation's first writer: it absorbs the WAR
        # waits vs last generation's PE readers on a multi-wait-capable
        # engine, keeping the (1-wait-limited) xbar transposes to one dep.
        qtwt = qtw_pool.tile([P, ND * 512], BF16, tag="qtw", name=f"qtw{qb}")
        nc.vector.memset(qtwt[:], 0.0)
        qtw3 = qtwt[:].rearrange("p (d i) -> p d i", i=512)
        for cq in range(4):
            sb = load_cast(q, qb * 4 + cq, "q")
            for hh in range(2):
                nc.sync.dma_start(
                    out=qtw3[:, hh * ND // 2:(hh + 1) * ND // 2,
                             cq * P:(cq + 1) * P],
                    in_=sb[:, hh * e // 2:(hh + 1) * e // 2], transpose=True)
        qtws[qb] = qtwt

    # SWDGE FIFO order is execution order for loads: K first (gates all QK),
    # first masks interleaved mid-K, first two Q windows next, then V (needed
    # at first PV, ~35us in), then W (first FC, ~50us in).
    for c in range(NK):
        sb = load_cast(k, c, "k")
        for hh in range(2):
            nc.sync.dma_start(out=kt_out(c, hh * ND // 2, (hh + 1) * ND // 2),
                              in_=sb[:, hh * e // 2:(hh + 1) * e // 2],
                              transpose=True)
        if c == 7:
            load_mask(0)
    prepare_qtw(0)
    load_mask(1)
    if QB > 1:
        prepare_qtw(1)
    for c in range(NK):
        nc.gpsimd.dma_start(out=VN[c][:], in_=v[c * P:(c + 1) * P, :])
    for c in range(ND):
        sb = load_cast(wout, c, "w")
        for hh in range(2):
            nc.sync.dma_start(out=wt_out(c, hh * ND // 2, (hh + 1) * ND // 2),
                              in_=sb[:, hh * e // 2:(hh + 1) * e // 2],
                              transpose=True)
    bout_bcast = bass.AP(tensor=bout.tensor, offset=bout.offset,
                         ap=[[0, P]] + list(bout.ap))
    nc.gpsimd.dma_start(out=bb[:], in_=bout_bcast)

    def make_fc(qb, ats, recips):
        def fc():
            for qt in range(4):
                qtg = qb * 4 + qt
                osb = opool.tile([P, e], FP32, tag="osb", name=f"osb{qtg}")
                for eb in range(EB):
                    pso = ps_o.tile([P, 512], FP32, tag="ps_o",
                                    name=f"pso{qtg}_{eb}")
                    for d in range(ND):
                        nc.tensor.matmul(
                            pso[:], ats[d][:, qt * P:(qt + 1) * P],
                            WTa[:, d * e + eb * 512: d * e + (eb + 1) * 512],
                            start=(d == 0), stop=(d == ND - 1))
                    nc.scalar.activation(osb[:, eb * 512:(eb + 1) * 512], pso[:],
                                         AF.Copy, bias=0.0,
                                         scale=recips[qt][:, 0:1])
                nc.vector.tensor_add(osb[:], osb[:], bb[:])
                nc.gpsimd.dma_start(out=out[qtg * P:(qtg + 1) * P, :],
                                    in_=osb[:])
        return fc

    pend_fc = None
    for qb in range(QB):
        if qb not in qtws:
            prepare_qtw(qb)
        qtwt = qtws[qb]
        pta = ptpool.tile([P, NK * 512], BF16, tag="pta", name=f"pta{qb}")
        nc.vector.memset(pta[:], 0.0)
        pta3 = pta[:].rearrange("p (c i) -> p c i", i=512)
        recips = []
        for qt in range(4):
            qtg = qb * 4 + qt
            et = epool.tile([P, s], BF16, tag="e", name=f"e{qtg}")
            rs4 = small.tile([P, KB], FP32, tag=f"rs{qt}", name=f"rs{qtg}")
            for kb2 in range(KB // 2):
                pss = ps_s.tile([P, 1024], FP32, tag="ps_s", name=f"pss{qtg}_{kb2}")
                for h in range(2):
                    kb = kb2 * 2 + h
                    for d in range(ND):
                        nc.tensor.matmul(
                            pss[:, h * 512:(h + 1) * 512],
                            qtwt[:, d * 512 + qt * P: d * 512 + (qt + 1) * P],
                            KTa[:, d * s + kb * 512: d * s + (kb + 1) * 512],
                            start=(d == 0), stop=(d == ND - 1))
                nc.scalar.activation(et[:, kb2 * 1024:(kb2 + 1) * 1024], pss[:],
                                     AF.Exp, bias=0.0, scale=inv_scale,
                                     accum_out=rs4[:, kb2:kb2 + 1])
            rs1 = small.tile([P, 1], FP32, tag=f"rs1_{qt}", name=f"rs1_{qtg}")
            nc.vector.reduce_sum(rs1[:], rs4[:, 0:KB // 2], axis=mybir.AxisListType.X)
            rec = small.tile([P, 1], FP32, tag=f"rec{qt}", name=f"rec{qtg}")
            nc.vector.reciprocal(rec[:], rs1[:])
            recips.append(rec)
            # mask streams on SWDGE (cast to bf16); P = E*mask into a fresh
            # tile so the P'T transpose has a single DVE producer dep
            mt = mpool.tile([P, s], BF16, tag="m", name=f"m{qtg}")
            nc.gpsimd.dma_start(out=mt[:], in_=mask[qtg * P:(qtg + 1) * P, :])
            pt2 = ppool.tile([P, s], BF16, tag="p2", name=f"p2_{qtg}")
            nc.vector.tensor_mul(pt2[:], et[:], mt[:])
            for jj in range(4):
                nc.sync.dma_start(
                    out=pta3[:, jj * NK // 4:(jj + 1) * NK // 4,
                             qt * P:(qt + 1) * P],
                    in_=pt2[:, jj * s // 4:(jj + 1) * s // 4], transpose=True)
        # fc_out for the PREVIOUS q-block is emitted between this block's QK
        # and PV phases: its PE matmuls fill the stall while the last q-tile's
        # exp->mask->transpose chain completes (PE was 55% occupied without it)
        if pend_fc is not None:
            pend_fc()
        ats = [atpool.tile([P, 512], BF16, tag=f"at{d}", name=f"at_{qb}_{d}")
               for d in range(ND)]
        for d in range(ND):
            psa = ps_a.tile([P, 512], FP32, tag="ps_a", name=f"psa{qb}_{d}")
            for c in range(NK):
                nc.tensor.matmul(psa[:], VN[c][:, d * P:(d + 1) * P],
                                 pta[:, c * 512:(c + 1) * 512],
                                 start=(c == 0), stop=(c == NK - 1))
            nc.scalar.activation(ats[d][:], psa[:], AF.Copy, bias=0.0, scale=1.0)
        pend_fc = make_fc(qb, ats, recips)
    pend_fc()


_DMA_TYPES = ("InstDmaTransposeAnt", "InstDMACopy")


def _offload_hwdge_waits(nc):
    """walrus's per-instruction sync-wait slots are tiny (1 for DMA structs,
    ~2 for compute structs). Move excess waits onto ENGINE_NOPs spliced just
    before the instruction on the same engine stream — the sequencer blocks
    on the nops' waits in order, then issues the instruction; semantics
    unchanged."""
    eng_map = {"EngineType.SP": nc.sync, "EngineType.Activation": nc.scalar,
               "EngineType.Pool": nc.gpsimd, "EngineType.PE": nc.tensor,
               "EngineType.DVE": nc.vector}
    for bb in nc.main_func.blocks:
        insts = list(bb.instructions)
        out = []
        for ins in insts:
            si = getattr(ins, "sync_info", None)
            eng = eng_map.get(str(getattr(ins, "engine", None)))
            if si is not None and eng is not None and si.on_wait:
                cap = 1
                if len(si.on_wait) > cap:
                    keep = si.on_wait[:cap] if cap > 0 else []
                    excess = si.on_wait[cap:]
                    opc = nc.isa.Opcode.NEURON_ISA_TPB_OPCODE_NOP
                    for w in excess:
                        nop = eng._isa(opc, {})
                        nop.engine = ins.engine
                        nop.sync_info = mybir.SyncInfo(on_wait=[w], on_update=[])
                        nc.inst_map[nop.name] = nop
                        out.append(nop)
                    ins.sync_info.on_wait = list(keep)
            out.append(ins)
        bb.instructions[:] = out


def build(inv_scale_factor=32.0, s=S, e=E, repeat=1):
    nc = bass.Bass("TRN2", target_bir_lowering=False, debug=False,
                   num_devices=N_CORES)
    q = nc.dram_tensor("q", [s, e], FP32, kind="ExternalInput").ap()
    k = nc.dram_tensor("k", [s, e], FP32, kind="ExternalInput").ap()
    v = nc.dram_tensor("v", [s, e], FP32, kind="ExternalInput").ap()
    mask = nc.dram_tensor("mask", [s, s], FP32, kind="ExternalInput").ap()
    wout = nc.dram_tensor("wout", [e, e], FP32, kind="ExternalInput").ap()
    bout = nc.dram_tensor("bout", [e], FP32, kind="ExternalInput").ap()
    out = nc.dram_tensor("out", [s, e], FP32, kind="ExternalOutput").ap()
    with tile.TileContext(nc) as tc:
        for _ in range(repeat):
            with ExitStack() as ctx:
                emit(ctx, tc, q, k, v, mask, wout, bout, out,
                     1.0 / float(inv_scale_factor), s=s, e=e)
    _offload_hwdge_waits(nc)
    return nc


def make_in_maps(query, key, value, dropout_mask, Wout, bout):
    f32 = np.float32
    Wout = np.ascontiguousarray(Wout, dtype=f32)
    bvec = np.ascontiguousarray(bout, dtype=f32)
    return [{
        "q": np.ascontiguousarray(query[i], dtype=f32),
        "k": np.ascontiguousarray(key[i], dtype=f32),
        "v": np.ascontiguousarray(value[i], dtype=f32),
        "mask": np.ascontiguousarray(dropout_mask[i], dtype=f32),
        "wout": Wout,
        "bout": bvec,
    } for i in range(N_CORES)]


def run(inputs, trace=False, **trace_kwargs):
    nc = build(float(inputs.get("inv_scale_factor", 32)))
    in_maps = make_in_maps(inputs["query"], inputs["key"], inputs["value"],
                           inputs["dropout_mask"], inputs["Wout"], inputs["bout"])
    res = bass_utils.run_bass_kernel_spmd(
        nc, in_maps, core_ids=list(range(N_CORES)), trace=trace, **trace_kwargs)
    out = np.stack([np.asarray(res.results[i]["out"]) for i in range(N_CORES)])
    return out.astype(np.float32), res


def kernel(query, key, value, dropout_mask, Wout, bout, inv_scale_factor=32):
    out, _ = run(dict(query=query, key=key, value=value,
                      dropout_mask=dropout_mask, Wout=Wout, bout=bout,
                      inv_scale_factor=inv_scale_factor))
    return out
